# revision 15
# baseline (speedup 1.0000x reference)
"""Trainium2 Bass kernel for nn_Attention_2 (gnn_message_passing).

Pure data parallel over the batch/node dim B=32768: 8 NeuronCores each
process 4096 rows. The kernel is split into two decoupled phases so the
memory-roofline context stream never stalls:

  Phase 1 (prologue, ~15us): the whole softmax/gate chain for all 32
  row-tiles runs back-to-back from a single resident copy of
  source_distance, in a transposed layout ((h,j) on partitions, b on the
  free dim) so every reduction is a TensorEngine matmul against tiny
  host-built constants. Output: compact per-tile aggregation weights
  w4c [128, 32*128] bf16 (8KB/partition).

  Phase 2 (steady state): per 128-row tile, one DVE op expands w4c into
  a rotating block-diagonal stationary region, then 32 back-to-back
  matmuls stream the bf16 context (host-cast, halving HBM traffic)
  through the PE, accumulating into PSUM. The PE sees a dense
  LDW+MM stream -> HAM stays warm; the ctx DMA owns the sync HWDGE
  ring exclusively (consts/out stores are on the scalar ring).
"""

import sys

for _p in ("/opt/trn_rl_repo", "/root/.axon_site/_ro/trn_rl_repo"):
    if _p not in sys.path:
        sys.path.insert(0, _p)

from contextlib import ExitStack

import numpy as np

import concourse.bass as bass
import concourse.mybir as mybir
import concourse.tile as tile
from concourse import bacc
from concourse.bass_utils import run_bass_kernel_spmd

# Problem shape (hardcoded; kernel.py must be self-contained)
B, K, D, H = 32768, 32, 192, 4
NCORES = 8
ROWS = B // NCORES          # 4096 rows per core
P = 128                     # partitions / rows per tile
NT = ROWS // P              # 32 tiles per core
G = 4                       # rows per aggregation block (G*K == P)
NB = P // G                 # 32 blocks per tile
HK = H * K                  # 128
GS = 4                      # tiles per phase-1 chain group (512-col free dim)
SP = GS * P                 # 512
NREG = 3                    # rotating stationary regions
CTB_BUFS = 12               # context tile double-buffer depth
OG = 8                      # output tiles batched per store DMA

F32 = mybir.dt.float32
BF16 = mybir.dt.bfloat16
REGW = NB * (P + G)  # 4224: aggregation stationary-weight region width

_CACHE: dict = {}


def build_program(nt: int = NT):
    rows = nt * P
    nc = bacc.Bacc("TRN2", target_bir_lowering=False, debug=False, num_devices=NCORES)

    # Host-pretransposed inputs: sd as [K, rows] and ctx as [P, nt*NB*D] with
    # ctx_host[p, (t, j, d)] = context[b0(t) + 4j + p//K, p%K, d] — so every
    # per-tile DMA reads one contiguous 12KB run per partition. ctx is cast
    # to bf16 on the host, halving the HBM read (the memory roofline).
    F32R = mybir.dt.float32r
    sd_d = nc.dram_tensor("sd", [K, rows], BF16, kind="ExternalInput").ap()
    ctx_d = nc.dram_tensor("ctx", [P, nt * NB * D], BF16, kind="ExternalInput").ap()
    kern_r_d = nc.dram_tensor("kern_r", [K, HK], F32R, kind="ExternalInput").ap()
    biases_d = nc.dram_tensor("biases_c", [HK, 1], F32, kind="ExternalInput").ap()
    blkones_d = nc.dram_tensor("blkones", [HK, H], F32R, kind="ExternalInput").ap()
    e4_d = nc.dram_tensor("e4", [H, HK], F32R, kind="ExternalInput").ap()
    gd_d = nc.dram_tensor("gd", [HK, HK], F32R, kind="ExternalInput").ap()
    gatebh_d = nc.dram_tensor("gatebh", [HK, 1], F32, kind="ExternalInput").ap()
    hg4h_d = nc.dram_tensor("hg4h", [HK, P], F32R, kind="ExternalInput").ap()
    mask4_d = nc.dram_tensor("mask4", [P, P], F32, kind="ExternalInput").ap()
    # output batched OG tiles per store, bf16, host-decoded: [g, p, (q d)]
    out_d = nc.dram_tensor("out", [(nt // OG) * P, OG * D], BF16,
                           kind="ExternalOutput").ap()

    with tile.TileContext(nc) as tc, ExitStack() as ctx:
        consts = ctx.enter_context(tc.tile_pool(name="consts", bufs=1))
        ctbp = ctx.enter_context(tc.tile_pool(name="ctbp", bufs=CTB_BUFS))
        smallp = ctx.enter_context(tc.tile_pool(name="smallp", bufs=4))
        outp = ctx.enter_context(tc.tile_pool(name="outp", bufs=3))
        ps_mm = ctx.enter_context(tc.tile_pool(name="ps_mm", bufs=4, space="PSUM"))
        ps_out = ctx.enter_context(tc.tile_pool(name="ps_out", bufs=3, space="PSUM"))

        # consts + the full sd panel arrive on the scalar HWDGE ring so the
        # ctx stream owns the sync ring from t=0
        c_kern = consts.tile([K, HK], F32R)
        nc.scalar.dma_start(c_kern[:], kern_r_d)
        c_bias = consts.tile([HK, 1], F32)
        nc.scalar.dma_start(c_bias[:], biases_d)
        c_blk = consts.tile([HK, H], F32R)
        nc.scalar.dma_start(c_blk[:], blkones_d)
        c_e4 = consts.tile([H, HK], F32R)
        nc.scalar.dma_start(c_e4[:], e4_d)
        c_gd = consts.tile([HK, HK], F32R)
        nc.scalar.dma_start(c_gd[:], gd_d)
        c_gbh = consts.tile([HK, 1], F32)
        nc.scalar.dma_start(c_gbh[:], gatebh_d)
        c_hg = consts.tile([HK, P], F32R)
        nc.scalar.dma_start(c_hg[:], hg4h_d)
        c_mask = consts.tile([P, P], F32)
        nc.scalar.dma_start(c_mask[:], mask4_d)
        sd_all = consts.tile([K, rows], BF16)
        nc.scalar.dma_start(sd_all[:], sd_d)

        # compact per-tile aggregation weights, filled by phase 1
        w4c = consts.tile([P, nt * P], BF16)

        regions = []
        for ri in range(NREG):
            reg = consts.tile([P, REGW], BF16, name=f"agg_region{ri}")
            nc.gpsimd.memset(reg[:], 0.0)
            regions.append(reg)

        def region_write_view(reg):
            # [128, 32, 4] view hitting cols 136j + i (the live columns of
            # buffer j, which starts at col 132j)
            return reg[:].rearrange("p (j x) -> p j x", x=G)[:, 0:REGW // G:(P + 2 * G) // G, :]

        mview = c_mask[:].rearrange("p (j x) -> p j x", x=G)

        # ---- context stream: two HWDGE DMAs per 128-row tile so the PE
        # gets fresh chunks every ~2us and the HAM never re-throttles; the
        # two halves go on different HWDGE rings (sync + scalar) to double
        # the DMA issue rate ----
        HB = NB // 2 * D
        ctbs = []
        for t in range(nt):
            ctb = ctbp.tile([P, NB * D], BF16)
            base = t * NB * D
            nc.sync.dma_start(ctb[:, 0:HB], ctx_d[:, base:base + HB])
            nc.scalar.dma_start(ctb[:, HB:NB * D], ctx_d[:, base + HB:base + NB * D])
            ctbs.append(ctb)

        # ---- phase 1: softmax/gate chain for all tiles, 4 tiles a group ----
        assert nt % GS == 0
        for g in range(nt // GS):
            r0 = g * SP
            sd_t = sd_all[:, r0:r0 + SP]

            # simi_T = exp(-0.5 * sd^2) in [K, SP] layout
            sq = smallp.tile([K, SP], F32, tag="sm")
            nc.vector.tensor_mul(sq[:], sd_t, sd_t)
            simi_T = smallp.tile([K, SP], F32R, tag="sm")
            nc.scalar.activation(simi_T[:], sq[:],
                                 mybir.ActivationFunctionType.Exp, scale=-0.5)

            # logits_T[(h,j), b] then p = exp(logits + bias)
            logits_ps = ps_mm.tile([HK, SP], F32, tag="mm")
            nc.tensor.matmul(logits_ps[:], lhsT=c_kern[:], rhs=simi_T[:])
            p_t = smallp.tile([HK, SP], F32R, tag="sm")
            nc.scalar.activation(p_t[:], logits_ps[:],
                                 mybir.ActivationFunctionType.Exp, bias=c_bias[:])
            p_tf = p_t[:].bitcast(F32)

            # per-(h,b) softmax denominator and its reciprocal, broadcast back
            s_ps = ps_mm.tile([H, SP], F32, tag="mm")
            nc.tensor.matmul(s_ps[:], lhsT=c_blk[:], rhs=p_t[:])
            rs_f = smallp.tile([H, SP], F32, tag="sm")
            nc.vector.reciprocal_approx_fast(out=rs_f[:], in_=s_ps[:])
            rs = smallp.tile([H, SP], F32R, tag="sm")
            nc.vector.tensor_copy(rs[:], rs_f[:])
            sbc_ps = ps_mm.tile([HK, SP], F32, tag="mm")
            nc.tensor.matmul(sbc_ps[:], lhsT=c_e4[:], rhs=rs[:])
            w_t = smallp.tile([HK, SP], F32R, tag="sm")
            nc.vector.tensor_mul(w_t[:], p_tf, sbc_ps[:])

            # gate: sigmoid(x) = 0.5*(1+tanh(x/2)); the 0.5 is folded into hg4h
            gl_ps = ps_mm.tile([HK, SP], F32, tag="mm")
            nc.tensor.matmul(gl_ps[:], lhsT=c_gd[:], rhs=w_t[:])
            th = smallp.tile([HK, SP], F32, tag="sm")
            nc.scalar.activation(th[:], gl_ps[:],
                                 mybir.ActivationFunctionType.Tanh,
                                 bias=c_gbh[:], scale=0.5)
            gated2 = smallp.tile([HK, SP], F32R, tag="sm")
            nc.vector.scalar_tensor_tensor(
                out=gated2[:], in0=th[:], scalar=1.0, in1=w_t[:].bitcast(F32),
                op0=mybir.AluOpType.add, op1=mybir.AluOpType.mult)

            # head-combine (replicated 4x over row-groups), then block-mask
            # into the compact per-tile weight store
            wrep_ps = ps_mm.tile([P, SP], F32, tag="mm")
            nc.tensor.matmul(wrep_ps[:], lhsT=c_hg[:], rhs=gated2[:])
            for q in range(GS):
                t = g * GS + q
                wv = wrep_ps[:, q * P:(q + 1) * P].rearrange("p (j x) -> p j x", x=G)
                dv = w4c[:, t * P:(t + 1) * P].rearrange("p (j x) -> p j x", x=G)
                nc.vector.tensor_mul(dv, wv, mview)

        # ---- phase 2: pure aggregation loop, paced by the ctx stream ----
        out_sb = None
        for t in range(nt):
            reg = regions[t % NREG]
            srcv = w4c[:, t * P:(t + 1) * P].rearrange("p (j x) -> p j x", x=G)
            nc.vector.tensor_copy(region_write_view(reg), srcv)

            ctb = ctbs[t]
            out_ps = ps_out.tile([P, D], F32, tag="outps")
            for j in range(NB):
                nc.tensor.matmul(out_ps[:],
                                 lhsT=reg[:, (P + G) * j:(P + G) * j + P],
                                 rhs=ctb[:, j * D:(j + 1) * D],
                                 start=(j == 0), stop=(j == NB - 1))
            q = t % OG
            if q == 0:
                out_sb = outp.tile([P, OG * D], BF16)
            nc.vector.tensor_copy(out_sb[:, q * D:(q + 1) * D], out_ps[:])
            if q == OG - 1:
                # one big bf16 store per OG tiles on the scalar HWDGE ring
                # (1.5KB contiguous per partition) so stores can't head-of-line
                # block ctx loads on the sync ring
                g0 = (t // OG) * P
                nc.scalar.dma_start(out_d[g0:g0 + P, :], out_sb[:])

    nc.compile()
    return nc


def _softmax(x):
    e = np.exp(x - x.max())
    return e / e.sum()


def build_consts(kernels, biases, gate_W, gate_b, gate_weights, gate_bias):
    f32 = np.float32
    kern_r = np.ascontiguousarray(kernels.transpose(1, 0, 2).reshape(K, HK)).astype(f32)
    biases_c = np.ascontiguousarray(biases.reshape(HK, 1)).astype(f32)
    blkones = np.kron(np.eye(H), np.ones((K, 1))).astype(f32)
    e4 = np.kron(np.eye(H), np.ones((1, K))).astype(f32)
    gd = np.kron(np.eye(H), gate_W).astype(f32)
    gatebh = (0.5 * np.tile(gate_b, H)).reshape(HK, 1).astype(f32)
    hg = _softmax(np.asarray(gate_weights, np.float64) + np.asarray(gate_bias, np.float64))
    hg4h = np.kron((0.5 * hg)[:, None] @ np.ones((1, H)), np.eye(K)).astype(f32)
    mask4 = (np.arange(P)[:, None] // K == np.arange(P)[None, :] % G).astype(f32)
    return dict(kern_r=kern_r, biases_c=biases_c, blkones=blkones, e4=e4, gd=gd,
                gatebh=gatebh, hg4h=hg4h, mask4=mask4)


def run(inputs: dict, trace: bool = False, **kw):
    """inputs: full-size arrays keyed as in setup_inputs(). Returns (out, results)."""
    if "nc" not in _CACHE:
        _CACHE["nc"] = build_program()
    nc = _CACHE["nc"]

    import ml_dtypes

    sd = np.ascontiguousarray(np.asarray(inputs["source_distance"], np.float32))
    ctx = np.ascontiguousarray(np.asarray(inputs["context"], np.float32))
    consts = build_consts(
        np.asarray(inputs["kernels"], np.float32),
        np.asarray(inputs["biases"], np.float32),
        np.asarray(inputs["gate_W"], np.float32),
        np.asarray(inputs["gate_b"], np.float32),
        np.asarray(inputs["gate_weights"], np.float32),
        np.asarray(inputs["gate_bias"], np.float32),
    )

    in_maps = []
    for c in range(NCORES):
        b0 = c * ROWS
        # host-side layout transforms so every device DMA run is long+contiguous
        sd_c = np.ascontiguousarray(sd[b0:b0 + ROWS].T).astype(ml_dtypes.bfloat16)  # [K, ROWS]
        ctx_c = np.ascontiguousarray(
            ctx[b0:b0 + ROWS].reshape(NT, NB, P, D).transpose(2, 0, 1, 3)
        ).reshape(P, NT * NB * D).astype(ml_dtypes.bfloat16)
        m = {"sd": sd_c, "ctx": ctx_c}
        m.update(consts)
        in_maps.append(m)

    results = run_bass_kernel_spmd(nc, in_maps, core_ids=list(range(NCORES)),
                                   trace=trace, **kw)
    outs = []
    for c in range(NCORES):
        a = np.asarray(results.results[c]["out"]).astype(np.float32)
        a = a.reshape(NT // OG, P, OG, D).transpose(0, 2, 1, 3).reshape(ROWS, D)
        outs.append(a)
    out = np.concatenate(outs, axis=0)
    return out, results


def kernel(**inputs) -> np.ndarray:
    out, _ = run(inputs)
    return out


# revision 16
# speedup vs baseline: 1.3155x; 1.3155x over previous
"""Trainium2 Bass kernel for nn_Attention_2 (gnn_message_passing).

Pure data parallel over the batch/node dim B=32768: 8 NeuronCores each
process 4096 rows. The kernel is split into two decoupled phases so the
memory-roofline context stream never stalls:

  Phase 1 (prologue, ~15us): the whole softmax/gate chain for all 32
  row-tiles runs back-to-back from a single resident copy of
  source_distance, in a transposed layout ((h,j) on partitions, b on the
  free dim) so every reduction is a TensorEngine matmul against tiny
  host-built constants. Output: compact per-tile aggregation weights
  w4c [128, 32*128] bf16 (8KB/partition).

  Phase 2 (steady state): per 128-row tile, one DVE op expands w4c into
  a rotating block-diagonal stationary region, then 32 back-to-back
  matmuls stream the bf16 context (host-cast, halving HBM traffic)
  through the PE, accumulating into PSUM. The PE sees a dense
  LDW+MM stream -> HAM stays warm; the ctx DMA owns the sync HWDGE
  ring exclusively (consts/out stores are on the scalar ring).
"""

import sys

for _p in ("/opt/trn_rl_repo", "/root/.axon_site/_ro/trn_rl_repo"):
    if _p not in sys.path:
        sys.path.insert(0, _p)

from contextlib import ExitStack

import numpy as np

import concourse.bass as bass
import concourse.mybir as mybir
import concourse.tile as tile
from concourse import bacc
from concourse.bass_utils import run_bass_kernel_spmd

# Problem shape (hardcoded; kernel.py must be self-contained)
B, K, D, H = 32768, 32, 192, 4
NCORES = 8
ROWS = B // NCORES          # 4096 rows per core
P = 128                     # partitions / rows per tile
NT = ROWS // P              # 32 tiles per core
G = 4                       # rows per aggregation block (G*K == P)
NB = P // G                 # 32 blocks per tile
HK = H * K                  # 128
GS = 4                      # tiles per phase-1 chain group (512-col free dim)
SP = GS * P                 # 512
NREG = 3                    # rotating stationary regions
CTB_BUFS = 12               # context tile double-buffer depth
OG = 8                      # output tiles batched per store DMA

F32 = mybir.dt.float32
BF16 = mybir.dt.bfloat16
REGW = NB * (P + G)  # 4224: aggregation stationary-weight region width

_CACHE: dict = {}


def build_program(nt: int = NT):
    rows = nt * P
    nc = bacc.Bacc("TRN2", target_bir_lowering=False, debug=False, num_devices=NCORES)

    # Host-pretransposed inputs: sd as [K, rows] and ctx as [P, nt*NB*D] with
    # ctx_host[p, (t, j, d)] = context[b0(t) + 4j + p//K, p%K, d] — so every
    # per-tile DMA reads one contiguous 12KB run per partition. ctx is cast
    # to bf16 on the host, halving the HBM read (the memory roofline).
    F32R = mybir.dt.float32r
    sd_d = nc.dram_tensor("sd", [K, rows], BF16, kind="ExternalInput").ap()
    ctx_d = nc.dram_tensor("ctx", [P, nt * NB * D], BF16, kind="ExternalInput").ap()
    kern_r_d = nc.dram_tensor("kern_r", [K, HK], F32R, kind="ExternalInput").ap()
    biases_d = nc.dram_tensor("biases_c", [HK, 1], F32, kind="ExternalInput").ap()
    blkones_d = nc.dram_tensor("blkones", [HK, H], F32R, kind="ExternalInput").ap()
    e4_d = nc.dram_tensor("e4", [H, HK], F32R, kind="ExternalInput").ap()
    gd_d = nc.dram_tensor("gd", [HK, HK], F32R, kind="ExternalInput").ap()
    gatebh_d = nc.dram_tensor("gatebh", [HK, 1], F32, kind="ExternalInput").ap()
    hg4h_d = nc.dram_tensor("hg4h", [HK, P], F32R, kind="ExternalInput").ap()
    mask4_d = nc.dram_tensor("mask4", [P, P], F32, kind="ExternalInput").ap()
    # output batched OG tiles per store, bf16, host-decoded: [g, p, (q d)]
    out_d = nc.dram_tensor("out", [(nt // OG) * P, OG * D], BF16,
                           kind="ExternalOutput").ap()

    with tile.TileContext(nc) as tc, ExitStack() as ctx:
        consts = ctx.enter_context(tc.tile_pool(name="consts", bufs=1))
        ctbp = ctx.enter_context(tc.tile_pool(name="ctbp", bufs=CTB_BUFS))
        smallp = ctx.enter_context(tc.tile_pool(name="smallp", bufs=4))
        outp = ctx.enter_context(tc.tile_pool(name="outp", bufs=3))
        ps_mm = ctx.enter_context(tc.tile_pool(name="ps_mm", bufs=4, space="PSUM"))
        ps_out = ctx.enter_context(tc.tile_pool(name="ps_out", bufs=3, space="PSUM"))

        # consts + the full sd panel arrive on the scalar HWDGE ring so the
        # ctx stream owns the sync ring from t=0
        c_kern = consts.tile([K, HK], F32R)
        nc.scalar.dma_start(c_kern[:], kern_r_d)
        c_bias = consts.tile([HK, 1], F32)
        nc.scalar.dma_start(c_bias[:], biases_d)
        c_blk = consts.tile([HK, H], F32R)
        nc.scalar.dma_start(c_blk[:], blkones_d)
        c_e4 = consts.tile([H, HK], F32R)
        nc.scalar.dma_start(c_e4[:], e4_d)
        c_gd = consts.tile([HK, HK], F32R)
        nc.scalar.dma_start(c_gd[:], gd_d)
        c_gbh = consts.tile([HK, 1], F32)
        nc.scalar.dma_start(c_gbh[:], gatebh_d)
        c_hg = consts.tile([HK, P], F32R)
        nc.scalar.dma_start(c_hg[:], hg4h_d)
        c_mask = consts.tile([P, P], F32)
        nc.scalar.dma_start(c_mask[:], mask4_d)
        sd_all = consts.tile([K, rows], BF16)
        nc.scalar.dma_start(sd_all[:], sd_d)

        # compact per-tile aggregation weights, filled by phase 1
        w4c = consts.tile([P, nt * P], BF16)

        regions = []
        for ri in range(NREG):
            reg = consts.tile([P, REGW], BF16, name=f"agg_region{ri}")
            nc.gpsimd.memset(reg[:], 0.0)
            regions.append(reg)

        def region_write_view(reg):
            # [128, 32, 4] view hitting cols 136j + i (the live columns of
            # buffer j, which starts at col 132j)
            return reg[:].rearrange("p (j x) -> p j x", x=G)[:, 0:REGW // G:(P + 2 * G) // G, :]

        mview = c_mask[:].rearrange("p (j x) -> p j x", x=G)

        # ---- context stream: two HWDGE DMAs per 128-row tile so the PE
        # gets fresh chunks every ~2us and the HAM never re-throttles; the
        # two halves go on different HWDGE rings (sync + scalar) to double
        # the DMA issue rate ----
        HB = NB // 2 * D
        ctbs = []
        for t in range(nt):
            ctb = ctbp.tile([P, NB * D], BF16)
            base = t * NB * D
            nc.sync.dma_start(ctb[:, 0:HB], ctx_d[:, base:base + HB])
            nc.sync.dma_start(ctb[:, HB:NB * D], ctx_d[:, base + HB:base + NB * D])
            ctbs.append(ctb)

        # ---- phase 1: softmax/gate chain for all tiles, 4 tiles a group ----
        assert nt % GS == 0
        for g in range(nt // GS):
            r0 = g * SP
            sd_t = sd_all[:, r0:r0 + SP]

            # simi_T = exp(-0.5 * sd^2) in [K, SP] layout
            sq = smallp.tile([K, SP], F32, tag="sm")
            nc.vector.tensor_mul(sq[:], sd_t, sd_t)
            simi_T = smallp.tile([K, SP], F32R, tag="sm")
            nc.scalar.activation(simi_T[:], sq[:],
                                 mybir.ActivationFunctionType.Exp, scale=-0.5)

            # logits_T[(h,j), b] then p = exp(logits + bias)
            logits_ps = ps_mm.tile([HK, SP], F32, tag="mm")
            nc.tensor.matmul(logits_ps[:], lhsT=c_kern[:], rhs=simi_T[:])
            p_t = smallp.tile([HK, SP], F32R, tag="sm")
            nc.scalar.activation(p_t[:], logits_ps[:],
                                 mybir.ActivationFunctionType.Exp, bias=c_bias[:])
            p_tf = p_t[:].bitcast(F32)

            # per-(h,b) softmax denominator and its reciprocal, broadcast back
            s_ps = ps_mm.tile([H, SP], F32, tag="mm")
            nc.tensor.matmul(s_ps[:], lhsT=c_blk[:], rhs=p_t[:])
            rs_f = smallp.tile([H, SP], F32, tag="sm")
            nc.vector.reciprocal_approx_fast(out=rs_f[:], in_=s_ps[:])
            rs = smallp.tile([H, SP], F32R, tag="sm")
            nc.vector.tensor_copy(rs[:], rs_f[:])
            sbc_ps = ps_mm.tile([HK, SP], F32, tag="mm")
            nc.tensor.matmul(sbc_ps[:], lhsT=c_e4[:], rhs=rs[:])
            w_t = smallp.tile([HK, SP], F32R, tag="sm")
            nc.vector.tensor_mul(w_t[:], p_tf, sbc_ps[:])

            # gate: sigmoid(x) = 0.5*(1+tanh(x/2)); the 0.5 is folded into hg4h
            gl_ps = ps_mm.tile([HK, SP], F32, tag="mm")
            nc.tensor.matmul(gl_ps[:], lhsT=c_gd[:], rhs=w_t[:])
            th = smallp.tile([HK, SP], F32, tag="sm")
            nc.scalar.activation(th[:], gl_ps[:],
                                 mybir.ActivationFunctionType.Tanh,
                                 bias=c_gbh[:], scale=0.5)
            gated2 = smallp.tile([HK, SP], F32R, tag="sm")
            nc.vector.scalar_tensor_tensor(
                out=gated2[:], in0=th[:], scalar=1.0, in1=w_t[:].bitcast(F32),
                op0=mybir.AluOpType.add, op1=mybir.AluOpType.mult)

            # head-combine (replicated 4x over row-groups), then block-mask
            # into the compact per-tile weight store
            wrep_ps = ps_mm.tile([P, SP], F32, tag="mm")
            nc.tensor.matmul(wrep_ps[:], lhsT=c_hg[:], rhs=gated2[:])
            for q in range(GS):
                t = g * GS + q
                wv = wrep_ps[:, q * P:(q + 1) * P].rearrange("p (j x) -> p j x", x=G)
                dv = w4c[:, t * P:(t + 1) * P].rearrange("p (j x) -> p j x", x=G)
                nc.vector.tensor_mul(dv, wv, mview)

        # ---- phase 2: pure aggregation loop, paced by the ctx stream ----
        out_sb = None
        for t in range(nt):
            reg = regions[t % NREG]
            srcv = w4c[:, t * P:(t + 1) * P].rearrange("p (j x) -> p j x", x=G)
            nc.vector.tensor_copy(region_write_view(reg), srcv)

            ctb = ctbs[t]
            out_ps = ps_out.tile([P, D], F32, tag="outps")
            for j in range(NB):
                nc.tensor.matmul(out_ps[:],
                                 lhsT=reg[:, (P + G) * j:(P + G) * j + P],
                                 rhs=ctb[:, j * D:(j + 1) * D],
                                 start=(j == 0), stop=(j == NB - 1))
            q = t % OG
            if q == 0:
                out_sb = outp.tile([P, OG * D], BF16)
            nc.vector.tensor_copy(out_sb[:, q * D:(q + 1) * D], out_ps[:])
            if q == OG - 1:
                # one big bf16 store per OG tiles on the scalar HWDGE ring
                # (1.5KB contiguous per partition) so stores can't head-of-line
                # block ctx loads on the sync ring
                g0 = (t // OG) * P
                nc.scalar.dma_start(out_d[g0:g0 + P, :], out_sb[:])

    nc.compile()
    return nc


def _softmax(x):
    e = np.exp(x - x.max())
    return e / e.sum()


def build_consts(kernels, biases, gate_W, gate_b, gate_weights, gate_bias):
    f32 = np.float32
    kern_r = np.ascontiguousarray(kernels.transpose(1, 0, 2).reshape(K, HK)).astype(f32)
    biases_c = np.ascontiguousarray(biases.reshape(HK, 1)).astype(f32)
    blkones = np.kron(np.eye(H), np.ones((K, 1))).astype(f32)
    e4 = np.kron(np.eye(H), np.ones((1, K))).astype(f32)
    gd = np.kron(np.eye(H), gate_W).astype(f32)
    gatebh = (0.5 * np.tile(gate_b, H)).reshape(HK, 1).astype(f32)
    hg = _softmax(np.asarray(gate_weights, np.float64) + np.asarray(gate_bias, np.float64))
    hg4h = np.kron((0.5 * hg)[:, None] @ np.ones((1, H)), np.eye(K)).astype(f32)
    mask4 = (np.arange(P)[:, None] // K == np.arange(P)[None, :] % G).astype(f32)
    return dict(kern_r=kern_r, biases_c=biases_c, blkones=blkones, e4=e4, gd=gd,
                gatebh=gatebh, hg4h=hg4h, mask4=mask4)


def run(inputs: dict, trace: bool = False, **kw):
    """inputs: full-size arrays keyed as in setup_inputs(). Returns (out, results)."""
    if "nc" not in _CACHE:
        _CACHE["nc"] = build_program()
    nc = _CACHE["nc"]

    import ml_dtypes

    sd = np.ascontiguousarray(np.asarray(inputs["source_distance"], np.float32))
    ctx = np.ascontiguousarray(np.asarray(inputs["context"], np.float32))
    consts = build_consts(
        np.asarray(inputs["kernels"], np.float32),
        np.asarray(inputs["biases"], np.float32),
        np.asarray(inputs["gate_W"], np.float32),
        np.asarray(inputs["gate_b"], np.float32),
        np.asarray(inputs["gate_weights"], np.float32),
        np.asarray(inputs["gate_bias"], np.float32),
    )

    in_maps = []
    for c in range(NCORES):
        b0 = c * ROWS
        # host-side layout transforms so every device DMA run is long+contiguous
        sd_c = np.ascontiguousarray(sd[b0:b0 + ROWS].T).astype(ml_dtypes.bfloat16)  # [K, ROWS]
        ctx_c = np.ascontiguousarray(
            ctx[b0:b0 + ROWS].reshape(NT, NB, P, D).transpose(2, 0, 1, 3)
        ).reshape(P, NT * NB * D).astype(ml_dtypes.bfloat16)
        m = {"sd": sd_c, "ctx": ctx_c}
        m.update(consts)
        in_maps.append(m)

    results = run_bass_kernel_spmd(nc, in_maps, core_ids=list(range(NCORES)),
                                   trace=trace, **kw)
    outs = []
    for c in range(NCORES):
        a = np.asarray(results.results[c]["out"]).astype(np.float32)
        a = a.reshape(NT // OG, P, OG, D).transpose(0, 2, 1, 3).reshape(ROWS, D)
        outs.append(a)
    out = np.concatenate(outs, axis=0)
    return out, results


def kernel(**inputs) -> np.ndarray:
    out, _ = run(inputs)
    return out
